# revision 3
# baseline (speedup 1.0000x reference)
"""Trainium2 Bass kernel for nn_MultiHeadAttention (B=8, S=1024, D=768, H=12).

Sharding: data-parallel over batch — one batch element per NeuronCore (8 cores).
No collectives needed; gather is a host-side stack.

v2: all matmul operands in bf16 (host-prepped weights/inputs, on-device
activations), with the projection GEMMs interleaved into the ACT-bound
attention loop so the PE never idles (HAM stays warm):

  - Q^T/K^T in (D,S) feature-major layout; V in (S, 12*65) with a ones
    column per head so attn@V also yields the softmax denominator Z
    (bv folded into bo on the host: bo_eff = bo + Wo @ bv).
  - fused pass: V-projection tiles are produced per t-block and consumed
    immediately by attention(p=0, strip=0); K(1) projection woven in.
  - attention(p, strip): per t-block: scoresT = KT_h.T @ QT_h (row-packed
    head pair), E = exp(SCALE*scores) on ACT (the bottleneck engine),
    acc += V_aug.T @ E accumulated in PSUM; software-pipelined one
    iteration so PE never waits on ACT; K/O projection matmuls are
    emitted as filler between iterations.
  - normalize: oht = acc[0:64] * (1/Z) via DVE reciprocal + gpsimd
    partition broadcast, multiplying straight out of PSUM.
  - O = oht.T @ WoT + bo_eff per 128-row strip, DMA'd out as produced.
"""
import sys

sys.path.insert(0, "/opt/trn_rl_repo")

import numpy as np
import ml_dtypes

import concourse.bacc as bacc
import concourse.tile as tile
from concourse import mybir
from concourse.bass_utils import run_bass_kernel_spmd

B, S, D, H = 8, 1024, 768, 12
DH = D // H                       # 64
NP = H // 2                       # 6 head pairs == D/128 tiles
DVP = H * (DH + 1)                # 780: V padded width (65 per head)
SCALE = 1.0 / np.sqrt(np.float32(D))
NT = S // 128                     # 8 seq tiles of 128
ND = D // 128                     # 6 feature tiles of 128

F32 = mybir.dt.float32
BF16 = mybir.dt.bfloat16
Exp = mybir.ActivationFunctionType.Exp

_CACHE = {}


def _build_nc(loop_n=1):
    nc = bacc.Bacc("TRN2", target_bir_lowering=False, debug=False)

    d = {}
    for name, shape, dt in [
        ("xqt", (D, S), BF16), ("xkt", (D, S), BF16), ("xvt", (D, S), BF16),
        ("wqt", (D, D), BF16), ("wkt", (D, D), BF16),
        ("wvtp", (D, DVP), BF16), ("wot", (D, D), BF16),
        ("bqc", (128, ND), F32), ("bkc", (128, ND), F32), ("bor", (1, D), F32),
    ]:
        d[name] = nc.dram_tensor(name, shape, dt, kind="ExternalInput").ap()
    out_d = nc.dram_tensor("out", (S, D), F32, kind="ExternalOutput").ap()

    with tile.TileContext(nc) as tc:
        for _ in range(loop_n):
            _emit(nc, tc, d, out_d)
    nc.compile()
    return nc


def _emit(nc, tc, d, out_d):
    import contextlib

    ctx = contextlib.ExitStack()
    with ctx:
        wq_pool = ctx.enter_context(tc.tile_pool(name="wq", bufs=6))
        wk_pool = ctx.enter_context(tc.tile_pool(name="wk", bufs=6))
        wv_pool = ctx.enter_context(tc.tile_pool(name="wv", bufs=6))
        wo_pool = ctx.enter_context(tc.tile_pool(name="wo", bufs=6))
        xq_pool = ctx.enter_context(tc.tile_pool(name="xq", bufs=6))
        xk_pool = ctx.enter_context(tc.tile_pool(name="xk", bufs=6))
        xv_pool = ctx.enter_context(tc.tile_pool(name="xv", bufs=6))
        qk_pool = ctx.enter_context(tc.tile_pool(name="qk", bufs=12))
        v_pool = ctx.enter_context(tc.tile_pool(name="v", bufs=8))
        e_pool = ctx.enter_context(tc.tile_pool(name="e", bufs=4))
        oht_pool = ctx.enter_context(tc.tile_pool(name="oht", bufs=6))
        o_pool = ctx.enter_context(tc.tile_pool(name="o", bufs=2))
        r_pool = ctx.enter_context(tc.tile_pool(name="r", bufs=2))
        rb_pool = ctx.enter_context(tc.tile_pool(name="rb", bufs=2))
        const_pool = ctx.enter_context(tc.tile_pool(name="const", bufs=1))
        ps = ctx.enter_context(tc.tile_pool(name="ps", bufs=2, space="PSUM"))
        ps_acc = ctx.enter_context(
            tc.tile_pool(name="ps_acc", bufs=4, space="PSUM"))

        # ---- constants ----
        bq_t = const_pool.tile([128, ND], F32, name="bq_t")
        bk_t = const_pool.tile([128, ND], F32, name="bk_t")
        bo_bc = const_pool.tile([128, D], F32, name="bo_bc")
        nc.gpsimd.dma_start(bq_t[:], d["bqc"][:])
        nc.gpsimd.dma_start(bk_t[:], d["bkc"][:])
        nc.gpsimd.dma_start(bo_bc[:], d["bor"].to_broadcast((128, D)))

        def load_wx(wkey, wpool, wwidth, xkey, xpool):
            wt, xt = [], []
            for i in range(ND):
                w = wpool.tile([128, wwidth], BF16, name=f"{wkey}{i}", tag="w")
                nc.sync.dma_start(w[:], d[wkey][i * 128:(i + 1) * 128, :])
                x = xpool.tile([128, S], BF16, name=f"{xkey}{i}", tag="x")
                nc.scalar.dma_start(x[:], d[xkey][i * 128:(i + 1) * 128, :])
                wt.append(w)
                xt.append(x)
            return wt, xt

        # ---- Q/K/V inputs + weights (DMA issue order = consumption order)
        wq, xq = load_wx("wqt", wq_pool, D, "xqt", xq_pool)
        wk, xk = load_wx("wkt", wk_pool, D, "xkt", xk_pool)
        wv, xv = load_wx("wvtp", wv_pool, DVP, "xvt", xv_pool)

        qt_tiles = [None] * NP
        kt_tiles = [None] * NP

        def proj_qk(which, w_t, x_t, b_t, p, pools=None):
            """One 128-row slice of the Q/K projection: two 512-col halves."""
            ot = qk_pool.tile([128, S], BF16, name=f"{which}t{p}", tag="qk")
            for half in range(2):
                pp = ps_acc.tile([128, 512], F32, name=f"{which}ps{p}_{half}", tag="acc")
                csl = slice(half * 512, (half + 1) * 512)
                for di in range(ND):
                    nc.tensor.matmul(pp[:], w_t[di][:, p * 128:(p + 1) * 128],
                                     x_t[di][:, csl],
                                     start=di == 0, stop=di == ND - 1)
                nc.vector.tensor_scalar_add(ot[:, csl], pp[:], b_t[:, p:p + 1])
            return ot

        # ---- Q projections + K(0) ----
        for p in range(NP):
            qt_tiles[p] = proj_qk("q", wq, xq, bq_t, p)
        kt_tiles[0] = proj_qk("k", wk, xk, bk_t, 0)

        v_tiles = [None] * NT

        def v_proj(tb):
            """V projection for t-block tb into a (128, DVP) bf16 tile with
            ones columns; PSUM from the sc pool (alternates with sc tiles)."""
            pp = ps.tile([128, 1024], F32, name=f"vps{tb}", tag="ps")
            for di in range(ND):
                st, sp = di == 0, di == ND - 1
                lhs = xv[di][:, tb * 128:(tb + 1) * 128]
                nc.tensor.matmul(pp[:, 0:512], lhs, wv[di][:, 0:512],
                                 start=st, stop=sp)
                nc.tensor.matmul(pp[:, 512:DVP], lhs, wv[di][:, 512:DVP],
                                 start=st, stop=sp)
            vt = v_pool.tile([128, DVP], BF16, name=f"v{tb}", tag="v")
            nc.vector.tensor_copy(vt[:, 0:512], pp[:, 0:512])
            nc.vector.tensor_copy(vt[:, 512:DVP], pp[:, 512:DVP])
            v3 = vt[:].rearrange("p (h e) -> p h e", e=DH + 1)
            nc.vector.memset(v3[:, :, DH:DH + 1], 1.0)
            v_tiles[tb] = vt

        # ---- filler units: emitted piecewise inside attention passes ----
        def k_unit(p):
            """K projection for pair p as 4 chunks."""
            ot = qk_pool.tile([128, S], BF16, name=f"kt{p}", tag="qk")
            kt_tiles[p] = ot
            state = {}

            def mk_mm(half):
                def f():
                    pp = ps_acc.tile([128, 512], F32, name=f"kps{p}_{half}", tag="acc")
                    state[half] = pp
                    csl = slice(half * 512, (half + 1) * 512)
                    for di in range(ND):
                        nc.tensor.matmul(
                            pp[:], wk[di][:, p * 128:(p + 1) * 128],
                            xk[di][:, csl], start=di == 0, stop=di == ND - 1)
                return f

            def mk_ev(half):
                def f():
                    csl = slice(half * 512, (half + 1) * 512)
                    nc.vector.tensor_scalar_add(ot[:, csl], state[half][:],
                                                bk_t[:, p:p + 1])
                return f

            return [mk_mm(0), mk_ev(0), mk_mm(1), mk_ev(1)]

        def o_unit(stt):
            """Output projection for s-rows [stt*128, (stt+1)*128)."""
            ssl = slice(stt * 128, (stt + 1) * 128)
            o_t = o_pool.tile([128, D], F32, name=f"o{stt}", tag="o")
            state = {}
            widths = ((0, 512), (512, 768))

            def mk_mm(half):
                def f():
                    lo, hi = widths[half]
                    pp = ps_acc.tile([128, hi - lo], F32,
                                     name=f"ops{stt}_{half}", tag="acc")
                    state[half] = pp
                    for di in range(ND):
                        nc.tensor.matmul(pp[:], oht_tiles[di][:, ssl],
                                         wo[di][:, lo:hi],
                                         start=di == 0, stop=di == ND - 1)
                return f

            def mk_ev(half):
                def f():
                    lo, hi = widths[half]
                    nc.vector.tensor_add(o_t[:, lo:hi], state[half][:],
                                         bo_bc[:, lo:hi])
                    if half == 1:
                        nc.sync.dma_start(out_d[ssl, :], o_t[:])
                return f

            return [mk_mm(0), mk_ev(0), mk_mm(1), mk_ev(1)]

        oht_tiles = [
            oht_pool.tile([128, S], BF16, name=f"oht{p}", tag="oht") for p in range(NP)
        ]

        def attention(p, strip, unit=None, fuse_v=False):
            """8 t-block iterations; acc matmuls software-pipelined one
            iteration behind the exp; `unit` chunks woven at odd tb."""
            sl = slice(strip * 512, strip * 512 + 512)
            acc0 = ps_acc.tile([65, 512], F32, name=f"acc0_{p}_{strip}", tag="acc")
            acc1 = ps_acc.tile([65, 512], F32, name=f"acc1_{p}_{strip}", tag="acc")
            c0 = p * 2 * (DH + 1)
            prev = None

            def acc_mms(tb, et):
                st, sp = tb == 0, tb == NT - 1
                vt = v_tiles[tb]
                nc.tensor.matmul(acc0[:], vt[:, c0:c0 + DH + 1],
                                 et[:, 0:512], start=st, stop=sp)
                nc.tensor.matmul(acc1[:], vt[:, c0 + DH + 1:c0 + 2 * (DH + 1)],
                                 et[:, 512:1024], start=st, stop=sp)

            for tb in range(NT):
                if fuse_v:
                    v_proj(tb)
                sc = ps.tile([128, 1024], F32, name=f"sc{p}_{strip}_{tb}", tag="ps")
                tsl = slice(tb * 128, (tb + 1) * 128)
                nc.tensor.matmul(sc[:, 0:512], kt_tiles[p][0:64, tsl],
                                 qt_tiles[p][0:64, sl], start=True, stop=True)
                nc.tensor.matmul(sc[:, 512:1024], kt_tiles[p][64:128, tsl],
                                 qt_tiles[p][64:128, sl], start=True,
                                 stop=True)
                et = e_pool.tile([128, 1024], BF16, name=f"e{p}_{strip}_{tb}", tag="e")
                nc.scalar.activation(et[:], sc[:], Exp, scale=float(SCALE))
                if unit is not None and tb % 2 == 1:
                    unit[tb // 2]()
                if prev is not None:
                    acc_mms(*prev)
                prev = (tb, et)
            acc_mms(*prev)

            # normalize: oht_h = acc[0:64] * (1/Z), straight out of PSUM
            for h01, acc in ((0, acc0), (1, acc1)):
                r = r_pool.tile([1, 512], F32, name=f"r{p}{strip}{h01}", tag="r")
                nc.vector.reciprocal(r[:], acc[DH:DH + 1, :])
                rb = rb_pool.tile([64, 512], F32, name=f"rb{p}{strip}{h01}", tag="rb")
                nc.gpsimd.partition_broadcast(rb[:], r[:])
                nc.vector.tensor_mul(
                    oht_tiles[p][h01 * 64:(h01 + 1) * 64, sl],
                    acc[0:64, :], rb[:])

        # ---- fused V + attention(0,0) with K(1) woven in ----
        attention(0, 0, unit=k_unit(1), fuse_v=True)
        # ---- strip 0 with K projections as filler ----
        for p in range(1, NP):
            attention(p, 0, unit=k_unit(p + 1) if p + 1 < NP else None)

        wo = []
        for i in range(ND):
            t = wo_pool.tile([128, D], BF16, name=f"wot{i}", tag="w")
            nc.sync.dma_start(t[:], d["wot"][i * 128:(i + 1) * 128, :])
            wo.append(t)

        # ---- strip 1 with O projections (rows 0:512) as filler ----
        for p in range(NP):
            attention(p, 1, unit=o_unit(p - 1) if 1 <= p <= 4 else None)
        # ---- tail: O projections for rows 512:1024 ----
        for stt in range(4, NT):
            for step in o_unit(stt):
                step()


def _prep_in_maps(queries, keys, values, Wq, bq, Wk, bk, Wv, bv, Wo, bo):
    """Host-side prep: transpose/pad/cast to the kernel's dram layouts."""
    bf = ml_dtypes.bfloat16
    wvt = np.asarray(Wv, np.float32).T              # (D, D) = (di, do)
    wvtp = np.zeros((D, DVP), np.float32)
    for h in range(H):
        wvtp[:, h * (DH + 1):h * (DH + 1) + DH] = \
            wvt[:, h * DH:(h + 1) * DH]
    bo_eff = (np.asarray(bo, np.float32)
              + np.asarray(Wo, np.float32) @ np.asarray(bv, np.float32))
    shared = {
        "wqt": np.ascontiguousarray(np.asarray(Wq, np.float32).T).astype(bf),
        "wkt": np.ascontiguousarray(np.asarray(Wk, np.float32).T).astype(bf),
        "wvtp": wvtp.astype(bf),
        "wot": np.ascontiguousarray(np.asarray(Wo, np.float32).T).astype(bf),
        "bqc": np.ascontiguousarray(
            np.asarray(bq, np.float32).reshape(ND, 128).T),
        "bkc": np.ascontiguousarray(
            np.asarray(bk, np.float32).reshape(ND, 128).T),
        "bor": np.ascontiguousarray(bo_eff.reshape(1, D)),
    }
    queries = np.asarray(queries, np.float32)
    keys = np.asarray(keys, np.float32)
    values = np.asarray(values, np.float32)
    return [
        {"xqt": np.ascontiguousarray(queries[b].T).astype(bf),
         "xkt": np.ascontiguousarray(keys[b].T).astype(bf),
         "xvt": np.ascontiguousarray(values[b].T).astype(bf),
         **shared}
        for b in range(B)
    ]


def _get_nc():
    if "nc" not in _CACHE:
        _CACHE["nc"] = _build_nc()
    return _CACHE["nc"]


def kernel(queries, keys, values, Wq, bq, Wk, bk, Wv, bv, Wo, bo):
    in_maps = _prep_in_maps(queries, keys, values,
                            Wq, bq, Wk, bk, Wv, bv, Wo, bo)
    nc = _get_nc()
    res = run_bass_kernel_spmd(nc, in_maps, core_ids=list(range(B)))
    return np.stack([res.results[b]["out"] for b in range(B)], axis=0)


def run_traced(inputs, tmpdir=None):
    """Profiled single run; returns BassKernelResults with exec_time_ns."""
    in_maps = _prep_in_maps(**inputs)
    nc = _get_nc()
    return run_bass_kernel_spmd(nc, in_maps, core_ids=list(range(B)),
                                trace=True, tmpdir=tmpdir)


# revision 9
# speedup vs baseline: 1.2255x; 1.2255x over previous
"""Trainium2 Bass kernel for nn_MultiHeadAttention (B=8, S=1024, D=768, H=12).

Sharding: data-parallel over batch — one batch element per NeuronCore (8 cores).
No collectives needed; gather is a host-side stack.

v2: all matmul operands in bf16 (host-prepped weights/inputs, on-device
activations), with the projection GEMMs interleaved into the ACT-bound
attention loop so the PE never idles (HAM stays warm):

  - Q^T/K^T in (D,S) feature-major layout; V in (S, 12*65) with a ones
    column per head so attn@V also yields the softmax denominator Z
    (bv folded into bo on the host: bo_eff = bo + Wo @ bv).
  - fused pass: V-projection tiles are produced per t-block and consumed
    immediately by attention(p=0, strip=0); K(1) projection woven in.
  - attention(p, strip): per t-block: scoresT = KT_h.T @ QT_h (row-packed
    head pair), E = exp(SCALE*scores) on ACT (the bottleneck engine),
    acc += V_aug.T @ E accumulated in PSUM; software-pipelined one
    iteration so PE never waits on ACT; K/O projection matmuls are
    emitted as filler between iterations.
  - normalize: oht = acc[0:64] * (1/Z) via DVE reciprocal + gpsimd
    partition broadcast, multiplying straight out of PSUM.
  - O = oht.T @ WoT + bo_eff per 128-row strip, DMA'd out as produced.
"""
import sys

sys.path.insert(0, "/opt/trn_rl_repo")

import numpy as np
import ml_dtypes

import concourse.bacc as bacc
import concourse.tile as tile
from concourse import mybir
from concourse.bass_utils import run_bass_kernel_spmd

B, S, D, H = 8, 1024, 768, 12
DH = D // H                       # 64
NP = H // 2                       # 6 head pairs == D/128 tiles
DVP = H * (DH + 1)                # 780: V padded width (65 per head)
SCALE = 1.0 / np.sqrt(np.float32(D))
NT = S // 128                     # 8 seq tiles of 128
ND = D // 128                     # 6 feature tiles of 128

F32 = mybir.dt.float32
BF16 = mybir.dt.bfloat16
Exp = mybir.ActivationFunctionType.Exp

_CACHE = {}


def _build_nc(loop_n=1):
    nc = bacc.Bacc("TRN2", target_bir_lowering=False, debug=False)

    d = {}
    for name, shape, dt in [
        ("xqt", (D, S), BF16), ("xkt", (D, S), BF16), ("xvt", (D, S), BF16),
        ("wqt", (D, D), BF16), ("wkt", (D, D), BF16),
        ("wvtp", (D, DVP), BF16), ("wot", (D, D), BF16),
        ("bqc", (128, ND), F32), ("bkc", (128, ND), F32), ("bor", (1, D), F32),
    ]:
        d[name] = nc.dram_tensor(name, shape, dt, kind="ExternalInput").ap()
    out_d = nc.dram_tensor("out", (S, D), F32, kind="ExternalOutput").ap()

    with tile.TileContext(nc) as tc:
        for _ in range(loop_n):
            _emit(nc, tc, d, out_d)
    nc.compile()
    return nc


def _emit(nc, tc, d, out_d):
    import contextlib

    ctx = contextlib.ExitStack()
    with ctx:
        wq_pool = ctx.enter_context(tc.tile_pool(name="wq", bufs=6))
        wk_pool = ctx.enter_context(tc.tile_pool(name="wk", bufs=6))
        wv_pool = ctx.enter_context(tc.tile_pool(name="wv", bufs=6))
        wo_pool = ctx.enter_context(tc.tile_pool(name="wo", bufs=6))
        xq_pool = ctx.enter_context(tc.tile_pool(name="xq", bufs=6))
        xk_pool = ctx.enter_context(tc.tile_pool(name="xk", bufs=6))
        xv_pool = ctx.enter_context(tc.tile_pool(name="xv", bufs=6))
        qk_pool = ctx.enter_context(tc.tile_pool(name="qk", bufs=12))
        v_pool = ctx.enter_context(tc.tile_pool(name="v", bufs=8))
        e_pool = ctx.enter_context(tc.tile_pool(name="e", bufs=4))
        oht_pool = ctx.enter_context(tc.tile_pool(name="oht", bufs=6))
        o_pool = ctx.enter_context(tc.tile_pool(name="o", bufs=2))
        r_pool = ctx.enter_context(tc.tile_pool(name="r", bufs=2))
        rb_pool = ctx.enter_context(tc.tile_pool(name="rb", bufs=2))
        const_pool = ctx.enter_context(tc.tile_pool(name="const", bufs=1))
        ps = ctx.enter_context(tc.tile_pool(name="ps", bufs=2, space="PSUM"))
        ps_acc = ctx.enter_context(
            tc.tile_pool(name="ps_acc", bufs=4, space="PSUM"))

        def load_wx(wkey, wpool, wwidth, xkey, xpool):
            wt, xt = [], []
            for i in range(ND):
                w = wpool.tile([128, wwidth], BF16, name=f"{wkey}{i}", tag="w")
                nc.sync.dma_start(w[:], d[wkey][i * 128:(i + 1) * 128, :])
                x = xpool.tile([128, S], BF16, name=f"{xkey}{i}", tag="x")
                nc.sync.dma_start(x[:], d[xkey][i * 128:(i + 1) * 128, :])
                wt.append(w)
                xt.append(x)
            return wt, xt

        # ---- inputs + weights, all on the sync queue in consumption order
        # (its per-iteration tail is early, so in the looped NEFF the next
        # iteration's loads prefetch during this iteration's attention)
        wq, xq = load_wx("wqt", wq_pool, D, "xqt", xq_pool)

        bq_t = const_pool.tile([128, ND], F32, name="bq_t")
        bk_t = const_pool.tile([128, ND], F32, name="bk_t")
        bo_r = const_pool.tile([1, D], F32, name="bo_r")
        bo_bc = const_pool.tile([128, D], F32, name="bo_bc")
        nc.sync.dma_start(bq_t[:], d["bqc"][:])
        nc.sync.dma_start(bk_t[:], d["bkc"][:])
        nc.sync.dma_start(bo_r[:], d["bor"][:])
        nc.gpsimd.partition_broadcast(bo_bc[:], bo_r[:])

        wk, xk = load_wx("wkt", wk_pool, D, "xkt", xk_pool)
        wv, xv = load_wx("wvtp", wv_pool, DVP, "xvt", xv_pool)

        qt_tiles = [None] * NP
        kt_tiles = [None] * NP

        def proj_qk(which, w_t, x_t, b_t, p, pools=None):
            """One 128-row slice of the Q/K projection: two 512-col halves."""
            ot = qk_pool.tile([128, S], BF16, name=f"{which}t{p}", tag="qk")
            for half in range(2):
                pp = ps_acc.tile([128, 512], F32, name=f"{which}ps{p}_{half}", tag="acc")
                csl = slice(half * 512, (half + 1) * 512)
                for di in range(ND):
                    nc.tensor.matmul(pp[:], w_t[di][:, p * 128:(p + 1) * 128],
                                     x_t[di][:, csl],
                                     start=di == 0, stop=di == ND - 1)
                nc.vector.tensor_scalar_add(ot[:, csl], pp[:], b_t[:, p:p + 1])
            return ot

        # ---- Q projections + K(0) ----
        for p in range(NP):
            qt_tiles[p] = proj_qk("q", wq, xq, bq_t, p)
        kt_tiles[0] = proj_qk("k", wk, xk, bk_t, 0)

        v_tiles = [None] * NT

        def v_proj(tb):
            """V projection for t-block tb into a (128, DVP) bf16 tile with
            ones columns; PSUM from the sc pool (alternates with sc tiles)."""
            pp = ps.tile([128, 1024], F32, name=f"vps{tb}", tag="ps")
            for di in range(ND):
                st, sp = di == 0, di == ND - 1
                lhs = xv[di][:, tb * 128:(tb + 1) * 128]
                nc.tensor.matmul(pp[:, 0:512], lhs, wv[di][:, 0:512],
                                 start=st, stop=sp)
                nc.tensor.matmul(pp[:, 512:DVP], lhs, wv[di][:, 512:DVP],
                                 start=st, stop=sp)
            vt = v_pool.tile([128, DVP], BF16, name=f"v{tb}", tag="v")
            nc.vector.tensor_copy(vt[:, 0:512], pp[:, 0:512])
            nc.vector.tensor_copy(vt[:, 512:DVP], pp[:, 512:DVP])
            v3 = vt[:].rearrange("p (h e) -> p h e", e=DH + 1)
            nc.vector.memset(v3[:, :, DH:DH + 1], 1.0)
            v_tiles[tb] = vt

        # ---- filler units: emitted piecewise inside attention passes ----
        def k_unit(p):
            """K projection for pair p as 4 chunks."""
            ot = qk_pool.tile([128, S], BF16, name=f"kt{p}", tag="qk")
            kt_tiles[p] = ot
            state = {}

            def mk_mm(half):
                def f():
                    pp = ps_acc.tile([128, 512], F32, name=f"kps{p}_{half}", tag="acc")
                    state[half] = pp
                    csl = slice(half * 512, (half + 1) * 512)
                    for di in range(ND):
                        nc.tensor.matmul(
                            pp[:], wk[di][:, p * 128:(p + 1) * 128],
                            xk[di][:, csl], start=di == 0, stop=di == ND - 1)
                return f

            def mk_ev(half):
                def f():
                    csl = slice(half * 512, (half + 1) * 512)
                    nc.vector.tensor_scalar_add(ot[:, csl], state[half][:],
                                                bk_t[:, p:p + 1])
                return f

            return [mk_mm(0), mk_ev(0), mk_mm(1), mk_ev(1)]

        def o_unit(stt):
            """Output projection for s-rows [stt*128, (stt+1)*128)."""
            ssl = slice(stt * 128, (stt + 1) * 128)
            o_t = o_pool.tile([128, D], F32, name=f"o{stt}", tag="o")
            state = {}
            widths = ((0, 512), (512, 768))

            def mk_mm(half):
                def f():
                    lo, hi = widths[half]
                    pp = ps_acc.tile([128, hi - lo], F32,
                                     name=f"ops{stt}_{half}", tag="acc")
                    state[half] = pp
                    for di in range(ND):
                        nc.tensor.matmul(pp[:], oht_tiles[di][:, ssl],
                                         wo[di][:, lo:hi],
                                         start=di == 0, stop=di == ND - 1)
                return f

            def mk_ev(half):
                def f():
                    lo, hi = widths[half]
                    nc.vector.tensor_add(o_t[:, lo:hi], state[half][:],
                                         bo_bc[:, lo:hi])
                    if half == 1:
                        # early stores ride the (by-then idle) sync queue;
                        # tail stores ride the ACT queue after the last exp,
                        # keeping the next iteration's input prefetch unblocked
                        eng = nc.sync if stt < 4 else nc.scalar
                        eng.dma_start(out_d[ssl, :], o_t[:])
                return f

            return [mk_mm(0), mk_ev(0), mk_mm(1), mk_ev(1)]

        oht_tiles = [
            oht_pool.tile([128, S], BF16, name=f"oht{p}", tag="oht") for p in range(NP)
        ]

        def attention(p, strip, unit=None, fuse_v=False):
            """8 t-block iterations; acc matmuls software-pipelined one
            iteration behind the exp; `unit` chunks woven at odd tb."""
            sl = slice(strip * 512, strip * 512 + 512)
            acc0 = ps_acc.tile([65, 512], F32, name=f"acc0_{p}_{strip}", tag="acc")
            acc1 = ps_acc.tile([65, 512], F32, name=f"acc1_{p}_{strip}", tag="acc")
            c0 = p * 2 * (DH + 1)
            prev = None

            def acc_mms(tb, et):
                st, sp = tb == 0, tb == NT - 1
                vt = v_tiles[tb]
                nc.tensor.matmul(acc0[:], vt[:, c0:c0 + DH + 1],
                                 et[:, 0:512], start=st, stop=sp)
                nc.tensor.matmul(acc1[:], vt[:, c0 + DH + 1:c0 + 2 * (DH + 1)],
                                 et[:, 512:1024], start=st, stop=sp)

            for tb in range(NT):
                if fuse_v:
                    v_proj(tb)
                sc = ps.tile([128, 1024], F32, name=f"sc{p}_{strip}_{tb}", tag="ps")
                tsl = slice(tb * 128, (tb + 1) * 128)
                nc.tensor.matmul(sc[:, 0:512], kt_tiles[p][0:64, tsl],
                                 qt_tiles[p][0:64, sl], start=True, stop=True)
                nc.tensor.matmul(sc[:, 512:1024], kt_tiles[p][64:128, tsl],
                                 qt_tiles[p][64:128, sl], start=True,
                                 stop=True)
                et = e_pool.tile([128, 1024], BF16, name=f"e{p}_{strip}_{tb}", tag="e")
                nc.scalar.activation(et[:], sc[:], Exp, scale=float(SCALE))
                if unit is not None and tb % 2 == 1:
                    unit[tb // 2]()
                if prev is not None:
                    acc_mms(*prev)
                prev = (tb, et)
            acc_mms(*prev)

            # normalize: oht_h = acc[0:64] * (1/Z), straight out of PSUM
            for h01, acc in ((0, acc0), (1, acc1)):
                r = r_pool.tile([1, 512], F32, name=f"r{p}{strip}{h01}", tag="r")
                nc.vector.reciprocal(r[:], acc[DH:DH + 1, :])
                rb = rb_pool.tile([64, 512], F32, name=f"rb{p}{strip}{h01}", tag="rb")
                nc.gpsimd.partition_broadcast(rb[:], r[:])
                nc.vector.tensor_mul(
                    oht_tiles[p][h01 * 64:(h01 + 1) * 64, sl],
                    acc[0:64, :], rb[:])

        # ---- fused V + attention(0,0) with K(1) woven in ----
        attention(0, 0, unit=k_unit(1), fuse_v=True)
        # ---- strip 0 with K projections as filler ----
        for p in range(1, NP):
            attention(p, 0, unit=k_unit(p + 1) if p + 1 < NP else None)

        wo = []
        for i in range(ND):
            t = wo_pool.tile([128, D], BF16, name=f"wot{i}", tag="w")
            nc.sync.dma_start(t[:], d["wot"][i * 128:(i + 1) * 128, :])
            wo.append(t)

        # ---- strip 1 with O projections (rows 0:512) as filler ----
        for p in range(NP):
            attention(p, 1, unit=o_unit(p) if p <= 3 else None)
        # ---- tail: O projections for rows 512:1024 ----
        for stt in range(4, NT):
            for step in o_unit(stt):
                step()


def _prep_in_maps(queries, keys, values, Wq, bq, Wk, bk, Wv, bv, Wo, bo):
    """Host-side prep: transpose/pad/cast to the kernel's dram layouts."""
    bf = ml_dtypes.bfloat16
    wvt = np.asarray(Wv, np.float32).T              # (D, D) = (di, do)
    wvtp = np.zeros((D, DVP), np.float32)
    for h in range(H):
        wvtp[:, h * (DH + 1):h * (DH + 1) + DH] = \
            wvt[:, h * DH:(h + 1) * DH]
    bo_eff = (np.asarray(bo, np.float32)
              + np.asarray(Wo, np.float32) @ np.asarray(bv, np.float32))
    shared = {
        "wqt": np.ascontiguousarray(np.asarray(Wq, np.float32).T).astype(bf),
        "wkt": np.ascontiguousarray(np.asarray(Wk, np.float32).T).astype(bf),
        "wvtp": wvtp.astype(bf),
        "wot": np.ascontiguousarray(np.asarray(Wo, np.float32).T).astype(bf),
        "bqc": np.ascontiguousarray(
            np.asarray(bq, np.float32).reshape(ND, 128).T),
        "bkc": np.ascontiguousarray(
            np.asarray(bk, np.float32).reshape(ND, 128).T),
        "bor": np.ascontiguousarray(bo_eff.reshape(1, D)),
    }
    queries = np.asarray(queries, np.float32)
    keys = np.asarray(keys, np.float32)
    values = np.asarray(values, np.float32)
    return [
        {"xqt": np.ascontiguousarray(queries[b].T).astype(bf),
         "xkt": np.ascontiguousarray(keys[b].T).astype(bf),
         "xvt": np.ascontiguousarray(values[b].T).astype(bf),
         **shared}
        for b in range(B)
    ]


def _get_nc():
    if "nc" not in _CACHE:
        _CACHE["nc"] = _build_nc()
    return _CACHE["nc"]


def kernel(queries, keys, values, Wq, bq, Wk, bk, Wv, bv, Wo, bo):
    in_maps = _prep_in_maps(queries, keys, values,
                            Wq, bq, Wk, bk, Wv, bv, Wo, bo)
    nc = _get_nc()
    res = run_bass_kernel_spmd(nc, in_maps, core_ids=list(range(B)))
    return np.stack([res.results[b]["out"] for b in range(B)], axis=0)


def run_traced(inputs, tmpdir=None):
    """Profiled single run; returns BassKernelResults with exec_time_ns."""
    in_maps = _prep_in_maps(**inputs)
    nc = _get_nc()
    return run_bass_kernel_spmd(nc, in_maps, core_ids=list(range(B)),
                                trace=True, tmpdir=tmpdir)
